# revision 62
# baseline (speedup 1.0000x reference)
"""MDM denoiser (RoPE transformer) Trainium2 kernel.

Sharding: pure data-parallel over batch — each of the 8 NeuronCores runs the
full 8-layer transformer on 4 of the 32 sequences. No collectives.

On-device layout: activations are feature-major [d on partitions, tokens on
free dim]; every projection is then a natural matmul with the weight matrix
as the stationary (lhsT) operand in its original [K, N] orientation.
Matmul inputs are bf16 (fp32 PSUM accumulation); the residual stream, LN
statistics and softmax normalization stay fp32.

Keeping the tensor engine hot (the main perf lever on TRN2 — idle gaps
re-throttle the HAM clock gate to half speed):
- software pipelining: within U1 the attention of chunk c-1 runs between the
  LN-apply and QKV of chunk c, so DVE latency hides under matmuls; weights
  prefetch one layer ahead; LN1 stats for layer l+1 are computed during U2(l).
- LN: column sums via ones-row matmuls on a bf16 shadow of h; the stats
  chain runs on DVE (frees the PSUM banks without waiting on the ACT queue);
  rstd = exp(-0.5*ln(var+eps)) so it uses the SAME ACT table set as the
  attention exp — the only per-layer table loads are lnexp<->gelu (2/layer).
- RoPE: rot(q) is ONE [128,128] permutation matmul on the biased projection
  (rot lagged one tile behind the base GEMM), q' = qb*cos + rot(qb)*sin with
  the elementwise work split across DVE and Pool.
- softmax: scores [key,query] per head; exp on ACT straight out of PSUM; the
  denominator comes from an appended ones-column in the V operand; all 8
  heads' denominators are gathered into one [8,T] tile via tiny SBUF->SBUF
  DMAs, inverted with ONE ACT Ln+Exp pair, and broadcast per head-pair with
  a K=8 one-hot selector matmul.
- output bias enters as a K=1 ones-row contraction.
"""

import os
import sys

for _p in (
    "/root/.axon_site",
    "/root/.axon_site/_ro/trn_rl_repo",
    "/root/.axon_site/_ro/pypackages",
    "/opt/trn_rl_repo",
):
    if os.path.isdir(_p) and _p not in sys.path:
        sys.path.append(_p)

import ml_dtypes
import numpy as np

import concourse.bass as bass
import concourse.tile as tile
from concourse import mybir
from concourse.bass import ds, ts
from concourse.bass_utils import run_bass_kernel_spmd
from concourse.vector_clock import ScopedClock

BF16 = ml_dtypes.bfloat16
F32 = mybir.dt.float32
BF = mybir.dt.bfloat16

B, T, D_IN = 32, 512, 150
D, L, H = 512, 8, 8
HD = D // H          # 64
FF = 4 * D           # 2048
LLM, TXT = 512, 20
NCORES = 8
BL = B // NCORES     # 4 sequences per core
TOK = BL * T         # 2048 tokens per core
P = 128
KD = D // P          # 4 k-tiles over the model dim
KF = FF // P         # 16 k-tiles over the FF dim
NQ = D // P          # 4 dout tiles per 512-wide projection
EPS = 1e-5

Alu = mybir.AluOpType
Act = mybir.ActivationFunctionType


class _TileContext(tile.TileContext):
    """TileContext whose kernel-tail drain is compatible with this walrus.

    The pinned walrus rejects >1 sync wait on a NO_STRUCT instruction (the
    Tile tail Drain), so spread the tail waits one per SP nop instead.
    """

    def _drain_and_barrier(self, tick_clock, wait_clock):
        probe = self.nc.sync.nop()
        wait_clock.add_sem_waits(
            probe.ins, ScopedClock({None: tick_clock.global_clock})
        )
        si = probe.ins.sync_info
        waits = list(si.on_wait) if si is not None else []
        probe.ins.sync_info = mybir.SyncInfo(on_wait=waits[:1], on_update=[])
        for w in waits[1:]:
            n = self.nc.sync.nop()
            n.ins.sync_info = mybir.SyncInfo(on_wait=[w], on_update=[])
        self.nc.sync.drain()
        self.nc.all_engine_barrier()
        assert self.sems is not None
        popped = self.nc._tile_sem_poison_stack.pop()
        assert popped is self._sem_poison
        self.nc.clear_and_free_semaphores(list(self.sems.allocated().values()))
        self.nc.all_engine_barrier()


def _split_sync_waits(nc):
    """The pinned walrus encodes at most ONE sync wait per instruction;
    hoist extra waits onto preceding same-engine NOPs."""
    nid = 0
    for fn in nc.m.functions:
        for bb in fn.blocks:
            out = []
            for ins in bb.instructions:
                si = getattr(ins, "sync_info", None)
                if si is not None and len(si.on_wait) > 1:
                    waits = list(si.on_wait)
                    for w in waits[:-1]:
                        nop = mybir.InstNoOp(
                            name=f"I-sw{nid}", ins=[], outs=[]
                        )
                        nid += 1
                        nop.engine = ins.engine
                        nop.sync_info = mybir.SyncInfo(
                            on_wait=[w], on_update=[]
                        )
                        out.append(nop)
                    ins.sync_info = mybir.SyncInfo(
                        on_wait=[waits[-1]], on_update=list(si.on_update)
                    )
                out.append(ins)
            bb.instructions = out


# ---------------------------------------------------------------------------
# device program
# ---------------------------------------------------------------------------

def _build_nc():
    nc = bass.Bass(target_bir_lowering=False)

    # ---- DRAM tensors -----------------------------------------------------
    # per-core data
    x_fm = nc.dram_tensor("x_fm", [P, 2, TOK], BF, kind="ExternalInput")
    enc_fm = nc.dram_tensor("enc_fm", [P, KD, BL, TXT], F32, kind="ExternalInput")
    onehot = nc.dram_tensor("onehot", [P, 8, BL], F32, kind="ExternalInput")
    # replicated weights/constants
    pe_tab = nc.dram_tensor("pe_tab", [P, 8, D], F32, kind="ExternalInput")
    w_t1 = nc.dram_tensor("w_t1", [P, KD, D], BF, kind="ExternalInput")
    w_t2 = nc.dram_tensor("w_t2", [P, KD, D], BF, kind="ExternalInput")
    w_txt = nc.dram_tensor("w_txt", [P, KD, D], BF, kind="ExternalInput")
    w_in = nc.dram_tensor("w_in", [P, 2, D], BF, kind="ExternalInput")
    w_qkv = nc.dram_tensor("w_qkv", [L, P, KD, 2 * D], BF, kind="ExternalInput")
    perm_d = nc.dram_tensor("perm_d", [P, P], BF, kind="ExternalInput")
    w_v = nc.dram_tensor("w_v", [L, P, KD, D], BF, kind="ExternalInput")
    w_o = nc.dram_tensor("w_o", [L, P, KD, D], BF, kind="ExternalInput")
    w_1 = nc.dram_tensor("w_1", [L, P, KD, FF], BF, kind="ExternalInput")
    w_2 = nc.dram_tensor("w_2", [L, P, KF, D], BF, kind="ExternalInput")
    w_out = nc.dram_tensor("w_out", [P, KD, D_IN], BF, kind="ExternalInput")
    sel8_d = nc.dram_tensor("sel8_d", [8, (H // 2) * P], BF, kind="ExternalInput")
    cos_t = nc.dram_tensor("cos_t", [P, T], BF, kind="ExternalInput")
    sin_t = nc.dram_tensor("sin_t", [P, T], BF, kind="ExternalInput")
    # biases: blk cols = ln1g(0:4) ln2g(4:8) bqk(8:16) brot(16:24) bo(24:28)
    #         b1(28:44) b2(44:48)
    blk = nc.dram_tensor("blk", [L, P, 48], F32, kind="ExternalInput")
    b_v = nc.dram_tensor("b_v", [L, H * HD], BF, kind="ExternalInput")
    bt1_fm = nc.dram_tensor("bt1_fm", [P, 4], F32, kind="ExternalInput")
    bemb_fm = nc.dram_tensor("bemb_fm", [P, 4], F32, kind="ExternalInput")
    bout_r = nc.dram_tensor("bout_r", [1, D_IN], BF, kind="ExternalInput")
    out_d = nc.dram_tensor("out", [D_IN, TOK], F32, kind="ExternalOutput")

    from contextlib import ExitStack

    with _TileContext(nc) as tc, ExitStack() as ctx:
        ep = ctx.enter_context
        wts = ep(tc.tile_pool(name="wts", bufs=4))
        singles = ep(tc.tile_pool(name="singles", bufs=1))
        lnp = ep(tc.tile_pool(name="lnp", bufs=2))
        qkp = ep(tc.tile_pool(name="qk", bufs=2))
        vp = ep(tc.tile_pool(name="vp", bufs=2))
        opl = ep(tc.tile_pool(name="op", bufs=2))
        ppl = ep(tc.tile_pool(name="pp", bufs=2))
        ybp = ep(tc.tile_pool(name="yb", bufs=2))
        hbp = ep(tc.tile_pool(name="hb", bufs=1))
        gpl = ep(tc.tile_pool(name="gp", bufs=1))
        tmp = ep(tc.tile_pool(name="tmp", bufs=3))
        stat = ep(tc.tile_pool(name="stat", bufs=4))
        psA = ep(tc.tile_pool(name="psA", bufs=3, space="PSUM"))
        psBC = ep(tc.tile_pool(name="psBC", bufs=2, space="PSUM"))
        psO = ep(tc.tile_pool(name="psO", bufs=2, space="PSUM"))
        psS = ep(tc.tile_pool(name="psS", bufs=1, space="PSUM"))
        if True:
            # ---- constants ----
            cos_sb = singles.tile([P, T], BF)
            sin_sb = singles.tile([P, T], BF)
            nc.sync.dma_start(cos_sb[:], cos_t[:])
            nc.sync.dma_start(sin_sb[:], sin_t[:])
            ones_bf = singles.tile([P, 1], BF)
            nc.vector.memset(ones_bf[:], 1.0)
            # K=1 stationary ones row: broadcasts a [1, T] vector to all
            # 128 partitions via one matmul (PSUM destination).
            ones1 = singles.tile([1, P], BF)
            nc.vector.memset(ones1[:], 1.0)
            # sel8[:, h*P:(h+1)*P] is a [8,128] one-hot block selecting row h
            # of an [8,T] rhs and broadcasting it to all 128 partitions
            sel8 = singles.tile([8, (H // 2) * P], BF)
            nc.sync.dma_start(sel8[:], sel8_d[:])
            # rotate-half permutation (2 heads per 128-partition block),
            # applied as a stationary matmul operand
            perm_sb = singles.tile([P, P], BF)
            nc.sync.dma_start(perm_sb[:], perm_d[:])
            eps_sb = singles.tile([1, 1], F32)
            nc.vector.memset(eps_sb[:], EPS)
            bt1_sb = singles.tile([P, 4], F32)
            nc.sync.dma_start(bt1_sb[:], bt1_fm[:])
            bemb_sb = singles.tile([P, 4], F32)
            nc.sync.dma_start(bemb_sb[:], bemb_fm[:])

            # ---- conditioning: timestep PE -> MLP, text mean -> linear ----
            pe_sb = wts.tile([P, 8, D], F32, tag="wbig", bufs=3)
            nc.sync.dma_start(pe_sb[:], pe_tab[:])
            wt1_sb = wts.tile([P, KD, D], BF, tag="wsml", bufs=3)
            nc.sync.dma_start(wt1_sb[:], w_t1[:])
            wt2_sb = wts.tile([P, KD, D], BF, tag="wsml", bufs=3)
            nc.sync.dma_start(wt2_sb[:], w_t2[:])
            wtxt_sb = wts.tile([P, KD, D], BF, tag="wsml", bufs=3)
            nc.sync.dma_start(wtxt_sb[:], w_txt[:])
            oh_sb = singles.tile([P, 8, BL], F32)
            nc.sync.dma_start(oh_sb[:], onehot[:])
            enc_sb = singles.tile([P, KD, BL, TXT], F32)
            nc.sync.dma_start(enc_sb[:], enc_fm[:])

            # gather timestep PE rows via one-hot matmul (fp32)
            tpe_sb = singles.tile([P, KD, BL], BF)
            for dt in range(KD):
                acc = psS.tile([P, BL], F32, tag="psS")
                for o in range(8):
                    nc.tensor.matmul(
                        acc[:],
                        pe_sb[:, o, ts(dt, P)],
                        oh_sb[:, o, :],
                        start=(o == 0),
                        stop=(o == 7),
                    )
                nc.vector.tensor_copy(tpe_sb[:, dt, :], acc[:])

            # t1 = silu(pe @ W_t1 + b_t1)
            t1_sb = singles.tile([P, KD, BL], BF)
            for dt in range(KD):
                acc = psS.tile([P, BL], F32, tag="psS")
                for k in range(KD):
                    nc.tensor.matmul(
                        acc[:],
                        wt1_sb[:, k, ts(dt, P)],
                        tpe_sb[:, k, :],
                        start=(k == 0),
                        stop=(k == KD - 1),
                    )
                nc.scalar.activation(
                    t1_sb[:, dt, :], acc[:], Act.Silu, bias=bt1_sb[:, dt : dt + 1]
                )

            # text mean (sum; /TXT is folded into W_txt on host)
            encr = singles.tile([P, KD, BL], F32)
            for k in range(KD):
                nc.vector.reduce_sum(
                    encr[:, k, :], enc_sb[:, k, :, :], axis=mybir.AxisListType.X
                )
            encb = singles.tile([P, KD, BL], BF)
            nc.vector.tensor_copy(encb[:], encr[:])

            # emb = t1 @ W_t2 + txtsum @ (W_txt/TXT) + (b_t2 + b_txt + b_in)
            emb_sb = singles.tile([P, KD, BL], F32)
            for dt in range(KD):
                acc = psS.tile([P, BL], F32, tag="psS")
                for k in range(KD):
                    nc.tensor.matmul(
                        acc[:],
                        wt2_sb[:, k, ts(dt, P)],
                        t1_sb[:, k, :],
                        start=(k == 0),
                        stop=False,
                    )
                for k in range(KD):
                    nc.tensor.matmul(
                        acc[:],
                        wtxt_sb[:, k, ts(dt, P)],
                        encb[:, k, :],
                        start=False,
                        stop=(k == KD - 1),
                    )
                nc.vector.tensor_scalar(
                    emb_sb[:, dt, :],
                    acc[:],
                    bemb_sb[:, dt : dt + 1],
                    None,
                    Alu.add,
                )

            # ---- input projection: h = x @ W_in + emb (b_in inside emb) ----
            x_sb = wts.tile([P, 2, TOK], BF, tag="wbig", bufs=3)
            nc.sync.dma_start(x_sb[:], x_fm[:])
            win_sb = wts.tile([P, 2, D], BF, tag="wsml", bufs=3)
            nc.sync.dma_start(win_sb[:], w_in[:])
            h = singles.tile([P, KD, TOK], F32)
            for c in range(BL):
                cs = ds(c * T, T)
                for dt in range(KD):
                    acc = psA.tile([P, T], F32, tag="psA")
                    for k in range(2):
                        nc.tensor.matmul(
                            acc[:],
                            win_sb[:, k, ts(dt, P)],
                            x_sb[:, k, cs],
                            start=(k == 0),
                            stop=(k == 1),
                        )
                    nc.vector.tensor_scalar(
                        h[:, dt, cs], acc[:], emb_sb[:, dt, c : c + 1], None, Alu.add
                    )

            # ---- layers -------------------------------------------------

            def ln_sums(c):
                """Column sums/variance of h[:, :, chunk c]. Returns (nr, mv):
                nr [2,T] bf16 with row0 = -mean (row1 = rstd filled later by
                ln_rstd); mv [2,T] f32 with row0 = mean^2, row1 = var. Uses
                only Copy/Square on ACT — legal in every activation table set,
                so this can run in any phase."""
                cs = ds(c * T, T)
                # bf16 shadow of h via DVE copy (2x_2P mode), squared on the
                # otherwise-idle Pool engine; sums via 1-cycle/row matmuls
                hb = hbp.tile([P, KD, T], BF, tag="hb", bufs=1)
                nc.vector.tensor_copy(hb[:], h[:, :, cs])
                hsq = hbp.tile([P, KD, T], BF, tag="hsq")
                nc.gpsimd.tensor_mul(hsq[:], hb[:], hb[:])
                s_ps = psS.tile([1, T], F32, tag="psS")
                q_ps = psBC.tile([1, T], F32, tag="psBC")
                for k in range(KD):
                    nc.tensor.matmul(
                        s_ps[:], ones_bf[:], hb[:, k, :],
                        start=(k == 0), stop=(k == KD - 1),
                    )
                for k in range(KD):
                    nc.tensor.matmul(
                        q_ps[:], ones_bf[:], hsq[:, k, :],
                        start=(k == 0), stop=(k == KD - 1),
                    )
                # DVE consumes the psum sums immediately (releases the psS
                # and psBC banks without waiting behind the ACT queue); the
                # squaring of the mean runs on ACT from SBUF (Square is in
                # every table set)
                negm_bf = stat.tile([1, T], BF, tag="statbf", bufs=8)
                nc.vector.tensor_scalar_mul(negm_bf[:], s_ps[:], -1.0 / D)
                qsum = stat.tile([1, T], BF, tag="stat", bufs=1)
                nc.vector.tensor_scalar_mul(qsum[:], q_ps[:], 1.0 / D)
                m2 = stat.tile([1, T], BF, tag="m2", bufs=1)
                nc.scalar.activation(m2[:], negm_bf[:], Act.Square)
                var = stat.tile([1, T], F32, tag="var", bufs=4)
                nc.vector.tensor_sub(var[:], qsum[:], m2[:])
                return negm_bf, var

            def ln_rstd(nv):
                """rstd = exp(-0.5*ln(var+eps)) -> [1,T] bf16. Ln/Exp live in
                the natural_log_exp ACT table set (same as the attention exp),
                so call this only while that set is loaded (U1/U2 entry)."""
                negm_bf, var = nv
                lv = stat.tile([1, T], F32, tag="stat", bufs=1)
                nc.scalar.activation(lv[:], var[:], Act.Ln, bias=eps_sb[:])
                rstd_bf = stat.tile([1, T], BF, tag="statbf", bufs=8)
                nc.scalar.activation(rstd_bf[:], lv[:], Act.Exp, scale=-0.5)
                return negm_bf, rstd_bf

            def ln_stats(c):
                return ln_rstd(ln_sums(c))

            def ln_apply(c, negm_bf, rstd_bf, g_sb_col, yb_t):
                """yb_t = (h[chunk c] - mean) * rstd * g via broadcast matmuls
                + DVE; no ACT involvement."""
                cs = ds(c * T, T)
                negm_b = psBC.tile([P, T], F32, tag="psBC")
                nc.tensor.matmul(negm_b[:], ones1[:], negm_bf[:],
                                 start=True, stop=True)
                rstd_b = psBC.tile([P, T], F32, tag="psBC")
                nc.tensor.matmul(rstd_b[:], ones1[:], rstd_bf[:],
                                 start=True, stop=True)
                for k in range(KD):
                    cen = tmp.tile([P, T], BF, tag="tmpbf")
                    nc.vector.tensor_tensor(cen[:], h[:, k, cs], negm_b[:], Alu.add)
                    nc.vector.scalar_tensor_tensor(
                        yb_t[:, k, :], cen[:], g_sb_col[:, k : k + 1], rstd_b[:],
                        Alu.mult, Alu.mult,
                    )

            def layernorm(c, g_sb_col, yb_t):
                ln_apply(c, *ln_stats(c), g_sb_col, yb_t)

            # ---- per-layer weight fetch (one layer of prefetch) ----------
            def fetch_attn_w(l):
                d = {}
                d["wqkv"] = wts.tile([P, KD, 2 * D], BF, tag="wbig", bufs=3, name="wqkv_sb")
                nc.sync.dma_start(d["wqkv"][:], w_qkv[l])
                d["wv"] = wts.tile([P, KD, D], BF, tag="wsml", bufs=3, name="wv_sb")
                nc.sync.dma_start(d["wv"][:], w_v[l])
                d["wo"] = wts.tile([P, KD, D], BF, tag="wsml", bufs=3, name="wo_sb")
                nc.sync.dma_start(d["wo"][:], w_o[l])
                d["blk"] = lnp.tile([P, 48], F32, tag="blk", name="blk_sb")
                nc.sync.dma_start(d["blk"][:], blk[l])
                # v bias enters the V GEMM as a K=1 ones-row contraction
                d["bv"] = lnp.tile([1, H * HD], BF, tag="bv", name="bv_sb")
                nc.sync.dma_start(d["bv"][:], b_v[l : l + 1])
                return d

            def fetch_ff_w(l, d):
                d["w1"] = wts.tile([P, KD, FF], BF, tag="wbig", bufs=3, name="w1_sb")
                nc.sync.dma_start(d["w1"][:], w_1[l])
                d["w2"] = wts.tile([P, KF, D], BF, tag="wbig", bufs=3, name="w2_sb")
                nc.sync.dma_start(d["w2"][:], w_2[l])

            # ---- U1 building blocks -------------------------------------
            def u1_qkv(c, wl, ybt):
                """q/k + rope for chunk c -> (q_sb, k_sb). The rotate-half
                permutation is ONE [128,128] matmul on the biased projection
                (qb = base + bias), so rot(qb) already carries the rotated
                bias; q' = qb*cos + rot(qb)*sin."""
                blk_sb = wl["blk"]
                bqk = blk_sb[:, 8:16]
                q_sb = qkp.tile([P, KD, T], BF, tag="q")
                k_sb = qkp.tile([P, KD, T], BF, tag="k")
                def _rope_finish(i, qb):
                    dst = q_sb if i < KD else k_sb
                    di = i % KD
                    rot = psA.tile([P, T], F32, tag="psA")
                    nc.tensor.matmul(rot[:], perm_sb[:], qb[:],
                                     start=True, stop=True)
                    nc.gpsimd.tensor_mul(dst[:, di, :], qb[:], cos_sb[:])
                    tb = tmp.tile([P, T], BF, tag="tmpbf")
                    nc.vector.tensor_tensor(tb[:], rot[:], sin_sb[:], Alu.mult)
                    nc.gpsimd.tensor_add(dst[:, di, :], dst[:, di, :], tb[:])

                pend = None
                for i in range(2 * KD):  # 0..3 q tiles, 4..7 k tiles
                    base = psA.tile([P, T], F32, tag="psA")
                    for k in range(KD):
                        nc.tensor.matmul(
                            base[:], wl["wqkv"][:, k, ds(i * P, P)], ybt[:, k, :],
                            start=(k == 0), stop=(k == KD - 1),
                        )
                    qb = tmp.tile([P, T], BF, tag="tmpbf")
                    nc.vector.tensor_scalar(
                        qb[:], base[:], bqk[:, i : i + 1], None, Alu.add
                    )
                    if pend is not None:
                        _rope_finish(*pend)
                    pend = (i, qb)
                _rope_finish(*pend)
                return q_sb, k_sb

            def u1_v(c, wl, ybt):
                """v for chunk c, token-major, bias + appended ones column."""
                v_aug = vp.tile([P, KD, H, HD + 1], BF, tag="v")
                nc.vector.memset(v_aug[:, :, :, HD : HD + 1], 1.0)
                for q in range(KD):  # token sub-tile within the chunk
                    acc = psA.tile([P, T], F32, tag="psA")
                    for k in range(KD):
                        nc.tensor.matmul(
                            acc[:], ybt[:, k, ds(q * P, P)], wl["wv"][:, k, :],
                            start=(k == 0), stop=False,
                        )
                    nc.tensor.matmul(
                        acc[:], ones1[:], wl["bv"][:],
                        start=False, stop=True,
                    )
                    nc.scalar.copy(
                        v_aug[:, q, :, 0:HD],
                        acc[:].rearrange("p (h d) -> p h d", h=H),
                    )
                return v_aug

            def u1_attn_core(c, q_sb, k_sb, v_aug):
                """scores/softmax-exp/PV for chunk c -> (o_sb unscaled, den8).
                Denominators of all 8 heads are gathered into one [8,T] tile
                via tiny SBUF->SBUF DMAs."""
                o_sb = opl.tile([P, KD, T], BF, tag="o")
                den8 = stat.tile([8, T], BF, tag="den", bufs=2)
                for hh in range(H):
                    rb = (hh % 2) * HD
                    col = hh // 2
                    qh = q_sb[rb : rb + HD, col, :]
                    kh = k_sb[rb : rb + HD, col, :]
                    p_t = ppl.tile([P, KD, T], BF, tag="p")
                    for kt in range(KD):
                        sc = psA.tile([P, T], F32, tag="psA")
                        nc.tensor.matmul(
                            sc[:], kh[:, ts(kt, P)], qh,
                            start=True, stop=True,
                        )
                        nc.scalar.activation(
                            p_t[:, kt, :], sc[:], Act.Exp, scale=0.125
                        )
                    po = psO.tile([HD + 1, T], F32, tag="psO")
                    for kt in range(KD):
                        nc.tensor.matmul(
                            po[:], v_aug[:, kt, hh, :], p_t[:, kt, :],
                            start=(kt == 0), stop=(kt == KD - 1),
                        )
                    dstg = stat.tile([1, T], BF, tag="dstg", bufs=2)
                    nc.vector.tensor_copy(dstg[:], po[HD : HD + 1, :])
                    nc.sync.dma_start(den8[hh : hh + 1, :], dstg[:])
                    nc.vector.tensor_copy(o_sb[rb : rb + HD, col, :],
                                          po[0:HD, :])
                return o_sb, den8

            def u1_attn_finish(c, o_sb, den8):
                """r = 1/den via one ACT Ln+Exp pair; head-PAIR broadcasts
                (each [128,T] covers the two heads sharing a column of o_sb)
                and in-place scaling."""
                lden = stat.tile([8, T], F32, tag="lden", bufs=1)
                nc.scalar.activation(lden[:], den8[:], Act.Ln)
                r8 = stat.tile([8, T], BF, tag="r8", bufs=2)
                nc.scalar.activation(r8[:], lden[:], Act.Exp, scale=-1.0)
                for pr in range(H // 2):
                    r_b = psBC.tile([P, T], F32, tag="psBC")
                    nc.tensor.matmul(r_b[:], sel8[:, ds(pr * P, P)], r8[:],
                                     start=True, stop=True)
                    nc.vector.tensor_tensor(
                        o_sb[:, pr, :], o_sb[:, pr, :], r_b[:], Alu.mult,
                    )
                return o_sb

            def u1_o(c, wl, o_sb):
                """out-projection + residual for chunk c."""
                cs = ds(c * T, T)
                bo_c = wl["blk"][:, 24:28]
                for dt in range(KD):
                    acc = psA.tile([P, T], F32, tag="psA")
                    for k in range(KD):
                        nc.tensor.matmul(
                            acc[:], wl["wo"][:, k, ts(dt, P)], o_sb[:, k, :],
                            start=(k == 0), stop=(k == KD - 1),
                        )
                    nc.vector.scalar_tensor_tensor(
                        h[:, dt, cs], acc[:], bo_c[:, dt : dt + 1], h[:, dt, cs],
                        Alu.add, Alu.add,
                    )

            # ---- prologue: LN1 sums for layer 0 -------------------------
            cur_w = fetch_attn_w(0)
            ln1_nv = [ln_sums(c) for c in range(BL)]
            ybt0_next = None

            for l in range(L):
                ln1g = cur_w["blk"][:, 0:4]
                ln2g = cur_w["blk"][:, 4:8]
                b1_c = cur_w["blk"][:, 28:44]
                b2_c = cur_w["blk"][:, 44:48]

                # ---- U1, software-pipelined: attention lags one chunk so
                # chunk c's ybt (DVE) computes under chunk c-1's attention ----
                fetch_ff_w(l, cur_w)
                ln1_st = [
                    ln_rstd(ln1_nv[c]) if (c > 0 or ybt0_next is None) else None
                    for c in range(BL)
                ]
                prev = None
                for c in range(BL):
                    if c == 0 and ybt0_next is not None:
                        ybt = ybt0_next
                    else:
                        ybt = ybp.tile([P, KD, T], BF, tag="yb")
                        ln_apply(c, *ln1_st[c], ln1g, ybt)
                    if prev is not None:
                        core_prev = u1_attn_core(prev[0], *prev[1])
                    qk = u1_qkv(c, cur_w, ybt)
                    if prev is not None:
                        o_prev = u1_attn_finish(prev[0], *core_prev)
                    v_aug = u1_v(c, cur_w, ybt)
                    if prev is not None:
                        u1_o(prev[0], cur_w, o_prev)
                    prev = (c, (*qk, v_aug))
                core_prev = u1_attn_core(prev[0], *prev[1])
                # LN2 stats for chunks 0..2 are ready (their attention
                # residuals landed earlier) — their sum matmuls give the PE
                # work to hide the last chunk's denominator chain
                ln2_st = [ln_stats(c) for c in range(BL - 1)]
                # precompute chunk 0's FF input so U2's first GEMM group
                # never waits on the LN-apply chain
                ygt0 = ybp.tile([P, KD, T], BF, tag="yb")
                ln_apply(0, *ln2_st[0], ln2g, ygt0)
                o_prev = u1_attn_finish(prev[0], *core_prev)
                u1_o(prev[0], cur_w, o_prev)

                # ---- U2: last LN2 stats, then FF per chunk (one switch to
                # the gelu set); LN1 sums for l+1 interleave a chunk behind --
                nxt_w = fetch_attn_w(l + 1) if l + 1 < L else None
                ln2_st.append(ln_stats(BL - 1))
                for c in range(BL):
                    cs = ds(c * T, T)
                    if c == 0:
                        ybt = ygt0
                    else:
                        ybt = ybp.tile([P, KD, T], BF, tag="yb")
                        ln_apply(c, *ln2_st[c], ln2g, ybt)
                    g_sb = gpl.tile([P, KF, T], BF, tag="g")
                    for dt in range(KF):
                        acc = psA.tile([P, T], F32, tag="psA")
                        for k in range(KD):
                            nc.tensor.matmul(
                                acc[:], cur_w["w1"][:, k, ts(dt, P)], ybt[:, k, :],
                                start=(k == 0), stop=(k == KD - 1),
                            )
                        nc.scalar.activation(
                            g_sb[:, dt, :], acc[:], Act.Gelu,
                            bias=b1_c[:, dt : dt + 1],
                        )
                    for dt in range(KD):
                        acc = psA.tile([P, T], F32, tag="psA")
                        for k in range(KF):
                            nc.tensor.matmul(
                                acc[:], cur_w["w2"][:, k, ts(dt, P)], g_sb[:, k, :],
                                start=(k == 0), stop=(k == KF - 1),
                            )
                        nc.vector.scalar_tensor_tensor(
                            h[:, dt, cs], acc[:], b2_c[:, dt : dt + 1], h[:, dt, cs],
                            Alu.add, Alu.add,
                        )
                    if l + 1 < L and c >= 1:
                        ln1_nv[c - 1] = ln_sums(c - 1)
                if l + 1 < L:
                    ln1_nv[BL - 1] = ln_sums(BL - 1)
                    # precompute chunk 0's ybt for layer l+1: the ln/exp
                    # table load this needs is one U1(l+1) does anyway
                    r0 = ln_rstd(ln1_nv[0])
                    ybt0_next = ybp.tile([P, KD, T], BF, tag="yb")
                    ln_apply(0, *r0, nxt_w["blk"][:, 0:4], ybt0_next)
                    cur_w = nxt_w

            # ---- output projection: bias enters as a K=1 ones-row
            # contraction; results DMA straight out of PSUM ----
            wout_sb = wts.tile([P, KD, D_IN], BF, tag="wsml", bufs=3)
            nc.sync.dma_start(wout_sb[:], w_out[:])
            bout_rs = singles.tile([1, D_IN], BF)
            nc.sync.dma_start(bout_rs[:], bout_r[:])
            ones_t = singles.tile([1, T], BF)
            nc.vector.memset(ones_t[:], 1.0)
            for c in range(BL):
                cs = ds(c * T, T)
                hbt = hbp.tile([P, KD, T], BF, tag="hb", bufs=1)
                nc.scalar.copy(hbt[:], h[:, :, cs])
                acc1 = psA.tile([P, T], F32, tag="psA")
                for k in range(KD):
                    nc.tensor.matmul(
                        acc1[:], wout_sb[:, k, 0:P], hbt[:, k, :],
                        start=(k == 0), stop=False,
                    )
                nc.tensor.matmul(
                    acc1[:], bout_rs[0:1, 0:P], ones_t[:],
                    start=False, stop=True,
                )
                o1 = tmp.tile([P, T], F32, tag="ob", bufs=2)
                nc.scalar.copy(o1[:], acc1[:])
                nc.sync.dma_start(out_d[0:P, cs], o1[:])
                acc2 = psA.tile([P, T], F32, tag="psA")
                for k in range(KD):
                    nc.tensor.matmul(
                        acc2[0 : D_IN - P, :], wout_sb[:, k, P:D_IN], hbt[:, k, :],
                        start=(k == 0), stop=False,
                    )
                nc.tensor.matmul(
                    acc2[0 : D_IN - P, :], bout_rs[0:1, P:D_IN], ones_t[:],
                    start=False, stop=True,
                )
                o2 = tmp.tile([P, T], F32, tag="ob", bufs=2)
                nc.scalar.copy(o2[0 : D_IN - P, :], acc2[0 : D_IN - P, :])
                nc.sync.dma_start(out_d[P:D_IN, cs], o2[0 : D_IN - P, :])

    _split_sync_waits(nc)
    return nc


# ---------------------------------------------------------------------------
# host-side preparation
# ---------------------------------------------------------------------------

def _fm(w):
    """[K, N] -> [128, K//128, N] (partition-major k-tiles)."""
    k, n = w.shape
    return np.ascontiguousarray(
        w.reshape(k // P, P, n).transpose(1, 0, 2)
    )


def _bias_fm(v):
    """[n*128] -> [128, n] feature-major per-partition columns."""
    return np.ascontiguousarray(v.reshape(-1, P).T)


def _rot_cols(w):
    """Fold the rotate-half permutation (with signs) into columns of a
    [K, D] q/k weight block: out col (h*64+d) = -w col (h*64+d+32) for d<32,
    +w col (h*64+d-32) otherwise."""
    w4 = w.reshape(w.shape[0], H, 2, HD // 2)
    out = np.empty_like(w4)
    out[:, :, 0, :] = -w4[:, :, 1, :]
    out[:, :, 1, :] = w4[:, :, 0, :]
    return out.reshape(w.shape)


def _rot_vec(v):
    v4 = v.reshape(H, 2, HD // 2)
    out = np.empty_like(v4)
    out[:, 0, :] = -v4[:, 1, :]
    out[:, 1, :] = v4[:, 0, :]
    return out.reshape(v.shape)


def _prep_shared(inputs):
    f32 = np.float32
    g = {}

    # rope tables: cos/sin [T, HD] -> feature-major [HD, T], rows duplicated
    inv = 1.0 / (10000.0 ** (np.arange(0, HD, 2, dtype=f32) / HD))
    ang = np.arange(T, dtype=f32)[:, None] * inv[None, :]
    ang = np.concatenate([ang, ang], axis=-1)          # [T, HD]
    cos = np.cos(ang).T.astype(f32)                    # [HD, T]
    sin = np.sin(ang).T.astype(f32)
    g["cos_t"] = np.concatenate([cos, cos], axis=0).astype(BF16)  # [128, T]
    # rotate-half permutation as a stationary matmul operand:
    # out[m] = -in[m+32] for m%64<32, +in[m-64+32...] i.e. +in[m-32] else
    perm = np.zeros((P, P), np.float32)
    for blk_i in range(2):
        b0 = blk_i * HD
        for m in range(HD):
            if m < HD // 2:
                perm[b0 + m + HD // 2, b0 + m] = -1.0
            else:
                perm[b0 + m - HD // 2, b0 + m] = 1.0
    g["perm_d"] = perm.astype(BF16)
    sel = np.zeros((H, (H // 2) * P), np.float32)
    for pr in range(H // 2):
        sel[2 * pr, pr * P : pr * P + HD] = 1.0
        sel[2 * pr + 1, pr * P + HD : (pr + 1) * P] = 1.0
    g["sel8_d"] = sel.astype(BF16)
    g["sin_t"] = np.concatenate([sin, sin], axis=0).astype(BF16)

    # timestep sinusoidal PE table for t in 0..1023 (t-major tiles)
    pos = np.arange(1024, dtype=f32)[:, None]
    div = np.exp(-np.log(10000.0) * np.arange(0, D, 2, dtype=f32) / D)
    a = pos * div[None, :]
    tab = np.stack([np.sin(a), np.cos(a)], axis=-1).reshape(1024, D).astype(f32)
    g["pe_tab"] = np.ascontiguousarray(tab.reshape(8, P, D).transpose(1, 0, 2))

    W_t1 = np.asarray(inputs["W_t1"], f32)
    W_t2 = np.asarray(inputs["W_t2"], f32)
    W_txt = np.asarray(inputs["W_txt"], f32)
    g["w_t1"] = _fm(W_t1).astype(BF16)
    g["w_t2"] = _fm(W_t2).astype(BF16)
    g["w_txt"] = _fm(W_txt / TXT).astype(BF16)
    g["bt1_fm"] = _bias_fm(np.asarray(inputs["b_t1"], f32))
    bemb = (
        np.asarray(inputs["b_t2"], f32)
        + np.asarray(inputs["b_txt"], f32)
        + np.asarray(inputs["b_in"], f32)
    )
    g["bemb_fm"] = _bias_fm(bemb)

    W_in = np.asarray(inputs["W_in"], f32)             # [150, 512]
    w_in_pad = np.zeros((2 * P, D), f32)
    w_in_pad[:D_IN] = W_in
    g["w_in"] = _fm(w_in_pad).astype(BF16)

    W_out = np.asarray(inputs["W_out"], f32)           # [512, 150]
    g["w_out"] = _fm(W_out).astype(BF16)
    g["bout_r"] = np.asarray(inputs["b_out"], f32).reshape(1, D_IN).astype(BF16)

    # per-layer weights
    Wqkv = np.asarray(inputs["Wqkv"], f32)             # [L, D, 3D]
    bqkv = np.asarray(inputs["bqkv"], f32)             # [L, 3D]
    ln1_g = np.asarray(inputs["ln1_g"], f32)
    ln1_b = np.asarray(inputs["ln1_b"], f32)
    ln2_g = np.asarray(inputs["ln2_g"], f32)
    ln2_b = np.asarray(inputs["ln2_b"], f32)
    Wo = np.asarray(inputs["Wo"], f32)
    bo = np.asarray(inputs["bo"], f32)
    W1 = np.asarray(inputs["W1"], f32)
    b1 = np.asarray(inputs["b1"], f32)
    W2 = np.asarray(inputs["W2"], f32)
    b2 = np.asarray(inputs["b2"], f32)

    w_qkv_l, w_v_l, w_o_l, w_1_l, w_2_l = [], [], [], [], []
    blk_l, bv_l = [], []
    for l in range(L):
        Wq = Wqkv[l][:, 0:D]
        Wk = Wqkv[l][:, D : 2 * D]
        Wv = Wqkv[l][:, 2 * D : 3 * D]
        bq_eff = bqkv[l][0:D] + ln1_b[l] @ Wq
        bk_eff = bqkv[l][D : 2 * D] + ln1_b[l] @ Wk
        bv_eff = bqkv[l][2 * D : 3 * D] + ln1_b[l] @ Wv
        qk_aug = np.concatenate([Wq, Wk], axis=1)       # [D, 2D]
        w_qkv_l.append(_fm(qk_aug).astype(BF16))
        w_v_l.append(_fm(Wv).astype(BF16))
        w_o_l.append(_fm(Wo[l]).astype(BF16))
        w_1_l.append(_fm(W1[l]).astype(BF16))
        w_2_l.append(_fm(W2[l]).astype(BF16))
        b1_eff = b1[l] + ln2_b[l] @ W1[l]
        bo_eff = bo[l]
        b2_eff = b2[l]
        bqk_fm = np.concatenate(
            [_bias_fm(bq_eff), _bias_fm(bk_eff)], axis=1
        )                                               # [128, 8]
        brot_fm = np.concatenate(
            [_bias_fm(_rot_vec(bq_eff)), _bias_fm(_rot_vec(bk_eff))], axis=1
        )
        blk_one = np.concatenate(
            [
                _bias_fm(ln1_g[l]),
                _bias_fm(ln2_g[l]),
                bqk_fm,
                brot_fm,
                _bias_fm(bo_eff),
                _bias_fm(b1_eff),
                _bias_fm(b2_eff),
            ],
            axis=1,
        )                                               # [128, 48]
        blk_l.append(blk_one)
        bv_l.append(bv_eff)
    g["w_qkv"] = np.stack(w_qkv_l)
    g["w_v"] = np.stack(w_v_l)
    g["w_o"] = np.stack(w_o_l)
    g["w_1"] = np.stack(w_1_l)
    g["w_2"] = np.stack(w_2_l)
    g["blk"] = np.stack(blk_l).astype(np.float32)
    g["b_v"] = np.stack(bv_l).astype(BF16)
    return g


def _prep_core(inputs, cc):
    f32 = np.float32
    d = {}
    bs = slice(cc * BL, (cc + 1) * BL)

    x = np.asarray(inputs["x"], f32)[bs]               # [4, 512, 150]
    x_t = x.reshape(TOK, D_IN).T                       # [150, 2048]
    x_pad = np.zeros((2 * P, TOK), f32)
    x_pad[:D_IN] = x_t
    d["x_fm"] = (
        x_pad.reshape(2, P, TOK).transpose(1, 0, 2).astype(BF16)
    )

    enc = np.asarray(inputs["enc_text"], f32)[bs]      # [4, 20, 512]
    enc_fm = enc.transpose(2, 0, 1)                    # [512, 4, 20]
    d["enc_fm"] = np.ascontiguousarray(
        enc_fm.reshape(KD, P, BL, TXT).transpose(1, 0, 2, 3)
    )

    tsv = np.asarray(inputs["timesteps"]).astype(np.int64)[bs]  # [4]
    oh = np.zeros((P, 8, BL), f32)
    for j, t in enumerate(tsv):
        oh[int(t) % P, int(t) // P, j] = 1.0
    d["onehot"] = oh
    return d


_CACHE = {}


def kernel(**inputs):
    if "nc" not in _CACHE:
        _CACHE["nc"] = _build_nc()
    nc = _CACHE["nc"]

    shared = _prep_shared(inputs)
    in_maps = []
    for cc in range(NCORES):
        m = dict(shared)
        m.update(_prep_core(inputs, cc))
        in_maps.append(m)

    res = run_bass_kernel_spmd(
        nc, in_maps, core_ids=list(range(NCORES)), **_CACHE.get("run_kwargs", {})
    )
    _CACHE["last_result"] = res

    outs = []
    for cc in range(NCORES):
        o = res.results[cc]["out"]                     # [150, 2048] f32
        outs.append(
            o.reshape(D_IN, BL, T).transpose(1, 2, 0)  # [4, 512, 150]
        )
    return np.ascontiguousarray(np.concatenate(outs, axis=0), dtype=np.float32)

